# revision 20
# baseline (speedup 1.0000x reference)
"""Causal multi-head attention (B=8, H=16, S=1024, D=64, fp32) on 8 TRN2 cores.

Sharding: the B*H = 128 independent attention instances are split 16 per
core (pure data parallel, no collectives).

Per-head algorithm (all on one core):
  - Load Q, K natural [S, D]; PE-transpose to Q^T, K^T [D, S].
  - Scores transposed: S^T[k, q] = (K^T_tile).T @ Q^T — contraction over
    d on 64 partitions, causal-skipped (only q >= k-tile-start columns).
  - P^T = exp(S^T / 8) straight out of PSUM on ScalarE (no max-subtraction:
    scores are O(1) so exp cannot overflow, and masked entries are exactly
    zeroed by multiplying the diagonal tile with an upper-triangular 0/1
    mask). The masked -10000 bias of the reference underflows to exactly 0
    after softmax, so the results agree.
  - O[q, :] = P^T.T @ [V | 1]: the appended ones column accumulates the
    softmax denominator; normalize with a per-row reciprocal on the way out.
"""

import numpy as np

import concourse.bass as bass
import concourse.mybir as mybir
import concourse.tile as tile
from concourse.bass_utils import run_bass_kernel_spmd
from concourse.masks import make_identity

B, H, S, D = 8, 16, 1024, 64
NCORES = 8
HPC = B * H // NCORES  # heads per core
P = 128
NQ = S // P
NK = S // P
F32 = mybir.dt.float32
F32R = mybir.dt.float32r

# float32r runs the PE at 1 cycle/row (vs 4 for float32) once the moving
# dim is >= 256. Verified against the fp32 reference before enabling.
USE_F32R_QK = True


def _score_chunks(w):
    """Split a width-w score row into PSUM-bank-sized pieces (<=512),
    keeping every piece >= 256 where possible (float32r full-rate)."""
    out = []
    while w > 512:
        take = 512 if (w - 512 >= 256 or w == 1024) else w - 256
        out.append(take)
        w -= take
    out.append(w)
    return out


def _attention_body(ctx_pools, tc, out, q, k, v):
    nc = tc.nc

    const = ctx_pools.enter_context(tc.tile_pool(name="const", bufs=1))
    io = ctx_pools.enter_context(tc.tile_pool(name="io", bufs=2))
    tp = ctx_pools.enter_context(tc.tile_pool(name="tp", bufs=2))
    ptp = ctx_pools.enter_context(tc.tile_pool(name="ptp", bufs=2))
    small = ctx_pools.enter_context(tc.tile_pool(name="small", bufs=4))
    obp = ctx_pools.enter_context(tc.tile_pool(name="obp", bufs=4))
    psum_t = ctx_pools.enter_context(tc.tile_pool(name="psum_t", bufs=3, space="PSUM"))
    psum_s = ctx_pools.enter_context(tc.tile_pool(name="psum_s", bufs=2, space="PSUM"))
    psum_o = ctx_pools.enter_context(tc.tile_pool(name="psum_o", bufs=2, space="PSUM"))

    ident = const.tile([P, P], F32)
    make_identity(nc, ident)

    qk_dt = F32R if USE_F32R_QK else F32

    for h in range(HPC):
        q_h = q[h].rearrange("(c p) d -> p c d", p=P)
        k_h = k[h].rearrange("(c p) d -> p c d", p=P)
        v_h = v[h].rearrange("(c p) d -> p c d", p=P)
        o_h = out[h].rearrange("(c p) d -> p c d", p=P)

        # Q, K loaded FLAT: partition p holds rows s in [8p, 8p+8) — fully
        # contiguous 2 KiB per partition, the cheapest DMA descriptor shape.
        # The transpose step below converts to natural-s-order Q^T/K^T.
        SPB = S // P  # seq rows per partition in the flat view (8)
        qf = io.tile([P, SPB, D], F32, tag="qf")
        nc.sync.dma_start(out=qf, in_=q[h].rearrange("(p x) d -> p x d", p=P))
        kf = io.tile([P, SPB, D], F32, tag="kf")
        nc.sync.dma_start(out=kf, in_=k[h].rearrange("(p x) d -> p x d", p=P))
        vp = io.tile([P, NK, D + 1], F32, tag="vp")
        nc.sync.dma_start(out=vp[:, :, 0:D], in_=v_h)
        nc.vector.memset(vp[:, :, D : D + 1], 1.0)

        # Q^T, K^T [64, 1024] in natural s order. Transposing flat d-slices
        # j, j+1 together ([128, 128] input) halves PE transpose work; each
        # output half holds columns s = 8p + j (stride-8 writes into qt/kt).
        # Tiles carry the matmul dtype (float32r needs pre-rounded values,
        # so the PSUM->SBUF copy performs the rounding cast).
        qt = tp.tile([D, S], qk_dt, tag="qt")
        kt = tp.tile([D, S], qk_dt, tag="kt")
        qt_v = qt.rearrange("d (p j) -> d p j", j=SPB)
        kt_v = kt.rearrange("d (p j) -> d p j", j=SPB)
        def _copy_act(out, in_):
            nc.scalar.copy(out=out, in_=in_)

        def _copy_dve(out, in_):
            nc.vector.tensor_copy(out=out, in_=in_)

        for src, dst, cp in ((qf, qt_v, _copy_dve), (kf, kt_v, _copy_dve)):
            for j in range(0, SPB, 2):
                ps = psum_t.tile([P, P], F32, tag="tpp")
                nc.tensor.transpose(
                    ps, src[:, j : j + 2, :].rearrange("p a d -> p (a d)"), ident
                )
                cp(dst[:, :, j], ps[0:D, :])
                cp(dst[:, :, j + 1], ps[D : 2 * D, :])

        # Phase A: P^T tiles per k-tile, exp'd and causal-masked.
        pts = []
        for ki in range(NK):
            w_all = S - ki * P
            pt = ptp.tile([P, w_all], F32, tag=f"pt{ki}")
            j0 = 0
            for w in _score_chunks(w_all):
                st = psum_s.tile([P, w], F32, tag="st")
                nc.tensor.matmul(
                    st,
                    lhsT=kt[:, ki * P : (ki + 1) * P],
                    rhs=qt[:, ki * P + j0 : ki * P + j0 + w],
                    start=True,
                    stop=True,
                )
                nc.scalar.activation(
                    out=pt[:, j0 : j0 + w],
                    in_=st,
                    func=mybir.ActivationFunctionType.Exp,
                    scale=0.125,
                )
                j0 += w
            # Zero the below-diagonal entries of the diagonal block on the
            # otherwise-idle GpSimd engine: keep where (q_rel - k_rel) >= 0.
            nc.gpsimd.affine_select(
                out=pt[:, 0:P],
                in_=pt[:, 0:P],
                compare_op=mybir.AluOpType.is_ge,
                fill=0.0,
                base=0,
                pattern=[[1, P]],
                channel_multiplier=-1,
            )
            pts.append(pt)

        # Phase B: O[q-tile] = sum_ki P^T_ki.T @ [V_ki | 1], then normalize
        # into a per-head staging tile so the store is a single DMA.
        oh = obp.tile([P, NQ, D], F32, tag="oh")
        for qi in range(NQ):
            ot = psum_o.tile([P, D + 1], F32, tag="ot")
            for ki in range(qi + 1):
                nc.tensor.matmul(
                    ot,
                    lhsT=pts[ki][:, (qi - ki) * P : (qi - ki + 1) * P],
                    rhs=vp[:, ki, :],
                    start=(ki == 0),
                    stop=(ki == qi),
                )
            rec = small.tile([P, 1], F32, tag="rec")
            nc.vector.reciprocal(rec, ot[:, D : D + 1])
            nc.scalar.mul(oh[:, qi, :], ot[:, 0:D], rec)
        nc.sync.dma_start(out=o_h, in_=oh)


def _split_sync_waits(nc, max_waits=1):
    """This walrus build rejects instructions with more than one sync wait.
    Move excess waits onto same-engine carrier drains placed just before."""
    counter = [0]

    def process_block(bb):
        new = []
        for inst in bb.instructions:
            for sub in getattr(inst, "blocks", []) or []:
                process_block(sub)
            si = inst.sync_info
            waits = list(si.on_wait) if (si and si.on_wait) else []
            if len(waits) > max_waits:
                while len(waits) > max_waits:
                    chunk, waits = waits[:max_waits], waits[max_waits:]
                    counter[0] += 1
                    new.append(
                        mybir.InstNoOp(
                            name=f"I-waitsplit-{counter[0]}",
                            engine=inst.engine,
                            sync_info=mybir.SyncInfo(on_wait=chunk, on_update=[]),
                        )
                    )
                inst.sync_info = mybir.SyncInfo(on_wait=waits, on_update=si.on_update)
            new.append(inst)
        bb.instructions = new

    for f in nc.m.functions:
        for bb in f.blocks:
            process_block(bb)


_NC_CACHE = {}


def _build(nrep=1):
    if nrep in _NC_CACHE:
        return _NC_CACHE[nrep]
    from contextlib import ExitStack

    nc = bass.Bass(trn_type="TRN2", target_bir_lowering=False, debug=False)
    q = nc.dram_tensor("q", [HPC, S, D], F32, kind="ExternalInput").ap()
    k = nc.dram_tensor("k", [HPC, S, D], F32, kind="ExternalInput").ap()
    v = nc.dram_tensor("v", [HPC, S, D], F32, kind="ExternalInput").ap()
    out = nc.dram_tensor("out", [HPC, S, D], F32, kind="ExternalOutput").ap()
    with tile.TileContext(nc) as tc:
        for _ in range(nrep):
            with ExitStack() as pools:
                _attention_body(pools, tc, out, q, k, v)
    _split_sync_waits(nc)
    _NC_CACHE[nrep] = nc
    return nc


def run(inputs, trace=False):
    """Run on 8 cores; returns (full_output, exec_time_ns_or_None)."""
    nc = _build()
    q = np.ascontiguousarray(np.asarray(inputs["q"], dtype=np.float32)).reshape(
        B * H, S, D
    )
    k = np.ascontiguousarray(np.asarray(inputs["k"], dtype=np.float32)).reshape(
        B * H, S, D
    )
    v = np.ascontiguousarray(np.asarray(inputs["v"], dtype=np.float32)).reshape(
        B * H, S, D
    )
    in_maps = [
        {
            "q": q[i * HPC : (i + 1) * HPC],
            "k": k[i * HPC : (i + 1) * HPC],
            "v": v[i * HPC : (i + 1) * HPC],
        }
        for i in range(NCORES)
    ]
    res = run_bass_kernel_spmd(nc, in_maps, list(range(NCORES)), trace=trace)
    full = np.concatenate([res.results[i]["out"] for i in range(NCORES)], axis=0)
    return full.reshape(B, H, S, D), res.exec_time_ns


def kernel(q, k, v):
    out, _ = run({"q": q, "k": k, "v": v})
    return out
